# revision 18
# baseline (speedup 1.0000x reference)
"""Trainium2 Bass kernel for DeductionNetworkSingleLayer.

Sharding: data-parallel over (batch, query-block). 8 cores; core c handles
batch b = c // 4, query rows [qb*512, (qb+1)*512) with qb = c % 4.
Each core computes the full network for its 512 query rows; no collectives.

Algebraic restructuring (all exact reassociations):
  - scoresT_h = (H wk_h^T q_h^T)^T is computed as H @ (wk_h^T q_h^T), so the
    per-head K projection over the full 2048-key sequence collapses into a
    256x512 "qw" matrix. bk drops out entirely (softmax shift-invariance
    over keys; only the query-side bias bq affects the distribution).
  - ctx_h = probs_h @ (A wv_h^T + bv) is computed as (probs_h @ [A|1|0]),
    with wv and wo merged into one per-head matrix wcomb_h = wo_h @ wv_h
    (built on-chip once per head), and the bv term reduced to the constant
    bias wo @ bv + bo applied to the accumulated A_m. The ones column of the
    augmented A yields the softmax denominator from the same matmul; the
    zero column pads N to an even count (fp32r requirement).

The head loop is software-pipelined: head h+1's q/qw/wcomb production is
emitted between head h's key-block loop and its tail, so the PE never waits
on the production's eviction chains. Prologue DMAs are staged with explicit
dependency edges so the first-needed chunks get full HBM bandwidth.

Host-side prep is pure layout marshalling (slicing / transposes / reshapes /
constant padding, no arithmetic).
"""

import os
import sys

import numpy as np

for _p in ("/opt/trn_rl_repo", os.path.expanduser("~/.axon_site/_ro/trn_rl_repo")):
    if _p not in sys.path and os.path.isdir(_p):
        sys.path.insert(0, _p)

import concourse.bass as bass
import concourse.mybir as mybir
import concourse.tile as tile
from concourse import bacc
from concourse.bass_utils import run_bass_kernel_spmd
from concourse.masks import make_identity
from concourse.tile import add_dep_helper

P = 128
B, SQ, SK = 2, 2048, 2048
E = 256          # embed dim == per-head key dim
S = 256          # src dim == per-head value dim
NH = 8
HID = 2 * S      # 512
NQ = 512         # query rows per core
NCORES = 8
EXP2_SHIFT = -90.0  # constant softmax shift for the raw-QK branch
F32 = mybir.dt.float32

LAST_RESULT = None


def _bcast_row(row_ap, parts=P):
    """AP that broadcasts a [1, N] DRAM row across `parts` partitions."""
    return bass.AP(
        tensor=row_ap.tensor,
        offset=row_ap.offset,
        ap=[[0, parts]] + list(row_ap.ap)[1:],
    )


def build_nc(mm_dtype_name: str | None = None):
    """Build the Bass program (same SPMD program for all 8 cores)."""
    MMDT = getattr(mybir.dt, mm_dtype_name or os.environ.get("BASS_MM_DTYPE", "float32r"))

    nc = bacc.Bacc("TRN2", target_bir_lowering=False, debug=False)

    di = lambda name, shape, dt=F32: nc.dram_tensor(name, shape, dt, kind="ExternalInput").ap()
    d_qt = di("qt", [E, NQ], MMDT)        # Q-shard transposed
    d_ht = di("ht", [E, SK], MMDT)        # H[b] transposed
    d_anat = di("anat", [SK, S + 2], MMDT)  # A[b] | ones | zeros
    d_wqt = di("wqt", [E, NH * E], MMDT)  # wq.T
    d_wkn = di("wkn", [NH * E, E], MMDT)  # wk (natural)
    d_wvn = di("wvn", [NH * S, S], MMDT)  # wv (natural)
    d_wot = di("wot", [NH * S, S], MMDT)  # wo.T
    d_w1t = di("w1t", [S, HID], MMDT)
    d_w2t = di("w2t", [HID, S], MMDT)
    d_bqc = di("bqc", [P, 16])            # bq as [128,16] column chunks
    d_bvz = di("bvz", [P, 16, 2], MMDT)   # bv col chunks | zeros (even-N rhs)
    d_boc = di("boc", [P, 2])
    d_b1c = di("b1c", [P, 4])
    d_b2c = di("b2c", [P, 2])
    d_gr = di("gr", [1, S])               # ln_g row
    d_br = di("br", [1, S])               # ln_b row
    d_scl = di("scl", [P, 1])             # attn_scale broadcast column
    d_out = nc.dram_tensor("out", [NQ, S], F32, kind="ExternalOutput").ap()

    with tile.TileContext(nc) as tc:
        from contextlib import ExitStack

        with ExitStack() as ctx:
            singles = ctx.enter_context(tc.tile_pool(name="singles", bufs=1))
            wts = ctx.enter_context(tc.tile_pool(name="wts", bufs=2))
            qthp = ctx.enter_context(tc.tile_pool(name="qthp", bufs=2))
            expp = ctx.enter_context(tc.tile_pool(name="expp", bufs=4))
            ctxp = ctx.enter_context(tc.tile_pool(name="ctxp", bufs=2))
            colsp = ctx.enter_context(tc.tile_pool(name="colsp", bufs=8))
            psA = ctx.enter_context(tc.tile_pool(name="psA", bufs=4, space="PSUM"))
            psB = ctx.enter_context(tc.tile_pool(name="psB", bufs=4, space="PSUM"))

            # -------- prologue loads; critical chunks first, rest dep-gated ----
            sb_qt = singles.tile([P, 2, NQ], MMDT, tag="qt")
            nc.sync.dma_start(sb_qt, d_qt.rearrange("(e p) n -> p e n", p=P))
            sb_ht = singles.tile([P, 2, SK], MMDT, tag="ht")
            ht_r = d_ht.rearrange("(e p) n -> p e n", p=P)
            ht_dmas = []
            for nb in range(4):
                ht_dmas.append(nc.sync.dma_start(
                    sb_ht[:, :, nb * 512 : (nb + 1) * 512],
                    ht_r[:, :, nb * 512 : (nb + 1) * 512],
                ))
            sb_anat = singles.tile([P, 16, S + 2], MMDT, tag="anat")
            an_r = d_anat.rearrange("(c p) s -> p c s", p=P)
            an_dmas = []
            for nb in range(4):
                an_dmas.append(nc.sync.dma_start(
                    sb_anat[:, nb * 4 : (nb + 1) * 4, :],
                    an_r[:, nb * 4 : (nb + 1) * 4, :],
                ))
            sb_w1t = singles.tile([P, 2, HID], MMDT, tag="w1t")
            dma_w1 = nc.sync.dma_start(sb_w1t, d_w1t.rearrange("(e p) n -> p e n", p=P))
            sb_w2t = singles.tile([P, 4, S], MMDT, tag="w2t")
            dma_w2 = nc.sync.dma_start(sb_w2t, d_w2t.rearrange("(t p) s -> p t s", p=P))

            sb_bqc = singles.tile([P, 16], F32, tag="bqc")
            nc.sync.dma_start(sb_bqc, d_bqc)
            sb_bvz = singles.tile([P, 16, 2], MMDT, tag="bvz")
            nc.sync.dma_start(sb_bvz, d_bvz)
            sb_boc = singles.tile([P, 2], F32, tag="boc")
            nc.sync.dma_start(sb_boc, d_boc)
            sb_b1c = singles.tile([P, 4], F32, tag="b1c")
            nc.sync.dma_start(sb_b1c, d_b1c)
            sb_b2c = singles.tile([P, 2], F32, tag="b2c")
            nc.sync.dma_start(sb_b2c, d_b2c)
            sb_scl = singles.tile([P, 1], F32, tag="scl")
            nc.sync.dma_start(sb_scl, d_scl)
            sb_g = singles.tile([P, S], F32, tag="gbc")
            nc.gpsimd.dma_start(sb_g, _bcast_row(d_gr[0:1, :]))
            sb_b = singles.tile([P, S], F32, tag="bbc")
            nc.gpsimd.dma_start(sb_b, _bcast_row(d_br[0:1, :]))

            ident = singles.tile([P, P], F32, tag="ident")
            make_identity(nc, ident)
            sb_n90 = singles.tile([P, 1], F32, tag="n90")
            nc.gpsimd.memset(sb_n90, EXP2_SHIFT)
            sb_eps = singles.tile([P, 1], F32, tag="eps")
            nc.gpsimd.memset(sb_eps, 1e-5)

            # bq pre-scaled by 1/16 (q-projection eviction computes (x+bq)/16)
            sb_bq16 = singles.tile([P, 16], F32, tag="bq16")
            nc.vector.tensor_scalar_mul(sb_bq16, sb_bqc, 1.0 / 16.0)
            sb_attn = singles.tile([P, 4, S], F32, tag="attn")
            sb_amt = singles.tile([P, 2, NQ], F32, tag="amt")
            nc.gpsimd.memset(sb_amt, 0.0)
            sb_ff1t = singles.tile([P, 4, NQ], MMDT, tag="ff1t")
            sb_boeff = singles.tile([P, 2], F32, tag="boeff")
            nc.vector.tensor_copy(sb_boeff, sb_boc)

            Exp = mybir.ActivationFunctionType.Exp
            Iden = mybir.ActivationFunctionType.Identity
            Relu = mybir.ActivationFunctionType.Relu
            Sqrt = mybir.ActivationFunctionType.Sqrt
            Square = mybir.ActivationFunctionType.Square
            SUB = mybir.AluOpType.subtract
            MUL = mybir.AluOpType.mult
            ADD = mybir.AluOpType.add

            wot_r = d_wot.rearrange("(t p) s -> p t s", p=P)
            wqt_r = d_wqt.rearrange("(e p) n -> p e n", p=P)
            wkn_r = d_wkn.rearrange("(t p) e -> p t e", p=P)
            wvn_r = d_wvn.rearrange("(t p) s -> p t s", p=P)

            def sc_exp(tag, c, lhs_tile, rhs_tile, bias, scale):
                """scoresT block c + exp eviction (two halves for latency)."""
                ps = psA.tile([P, NQ], F32, tag="work", name=f"scps_{tag}_{c}")
                mm0 = nc.tensor.matmul(
                    ps, lhs_tile[:, 0, c * P : (c + 1) * P], rhs_tile[:, 0, :],
                    start=True, stop=False,
                )
                nc.tensor.matmul(
                    ps, lhs_tile[:, 1, c * P : (c + 1) * P], rhs_tile[:, 1, :],
                    start=False, stop=True,
                )
                ex = expp.tile([P, NQ], MMDT, tag="exp", name=f"exp_{tag}_{c}")
                nc.scalar.activation(ex[:, 0:256], ps[:, 0:256], Exp, bias=bias, scale=scale)
                nc.scalar.activation(ex[:, 256:512], ps[:, 256:512], Exp, bias=bias, scale=scale)
                return ex, mm0

            def ctx_mms(c, ex, acc):
                for qb2 in range(4):
                    nc.tensor.matmul(
                        acc[qb2],
                        ex[:, qb2 * P : (qb2 + 1) * P],
                        sb_anat[:, c, :],
                        start=(c == 0),
                        stop=(c == 15),
                    )

            # ============ Branch 1: 8-head attention (software-pipelined) ========
            def head_dmas(h, gate=None):
                w = {}
                w["q"] = wts.tile([P, 2, E], MMDT, tag="wq", name=f"wqh{h}")
                d1 = nc.sync.dma_start(w["q"], wqt_r[:, :, h * E : (h + 1) * E])
                w["k"] = wts.tile([P, 2, E], MMDT, tag="wk", name=f"wkh{h}")
                d2 = nc.sync.dma_start(w["k"], wkn_r[:, h * 2 : h * 2 + 2, :])
                w["v"] = wts.tile([P, 2, S], MMDT, tag="wv", name=f"wvh{h}")
                d3 = nc.sync.dma_start(w["v"], wvn_r[:, h * 2 : h * 2 + 2, :])
                w["o"] = wts.tile([P, 2, S], MMDT, tag="wo", name=f"woh{h}")
                d4 = nc.sync.dma_start(w["o"], wot_r[:, h * 2 : h * 2 + 2, :])
                if gate is not None:
                    for d in (d1, d2, d3, d4):
                        add_dep_helper(d.ins, gate.ins)
                return w

            def produce(h, w):
                """qth, wct, qwt for head h (wct between the two dependent steps)."""
                sb_qth = qthp.tile([P, 2, NQ], MMDT, tag="qth", name=f"qth{h}")
                qps = []
                for eo in range(2):
                    ps = psA.tile([P, NQ], F32, tag="work", name=f"qps{h}_{eo}")
                    for ei in range(2):
                        nc.tensor.matmul(
                            ps,
                            w["q"][:, ei, eo * P : (eo + 1) * P],
                            sb_qt[:, ei, :],
                            start=(ei == 0), stop=(ei == 1),
                        )
                    qps.append(ps)
                # wcombT_h = wv_h^T @ wo_h^T (independent; fills the evict gap)
                sb_wct = ctxp.tile([P, 2, S], MMDT, tag="wct", name=f"wct{h}")
                for sb2 in range(2):
                    ps = psA.tile([P, NQ], F32, tag="work", name=f"wcps{h}_{sb2}")
                    for fc in range(2):
                        nc.tensor.matmul(
                            ps[:, 0:S],
                            w["v"][:, fc, sb2 * P : (sb2 + 1) * P],
                            w["o"][:, fc, :],
                            start=(fc == 0), stop=(fc == 1),
                        )
                    nc.scalar.copy(sb_wct[:, sb2, :], ps[:, 0:S])
                for eo in range(2):
                    nc.vector.tensor_scalar(
                        sb_qth[:, eo, :], qps[eo], 1.0 / 16.0,
                        sb_bq16[:, h * 2 + eo : h * 2 + eo + 1], MUL, ADD,
                    )
                sb_qwt = qthp.tile([P, 2, NQ], MMDT, tag="qwt", name=f"qwt{h}")
                for eo in range(2):
                    ps = psA.tile([P, NQ], F32, tag="work", name=f"qwps{h}_{eo}")
                    for fc in range(2):
                        nc.tensor.matmul(
                            ps,
                            w["k"][:, fc, eo * P : (eo + 1) * P],
                            sb_qth[:, fc, :],
                            start=(fc == 0), stop=(fc == 1),
                        )
                    nc.vector.tensor_copy(sb_qwt[:, eo, :], ps)
                return sb_qwt, sb_wct

            # ============ Branch 2: attn_out = softmax(Q H^T * scale) @ A ========
            att_ps = [psB.tile([P, S + 2], F32, tag="acc", name=f"attps{i}") for i in range(4)]
            b2mm = []
            _prod0 = {}
            pexp, m0 = sc_exp("b2", 0, sb_ht, sb_qt, sb_n90, sb_scl)
            b2mm.append(m0)
            for c in range(1, 16):
                ex, m0 = sc_exp("b2", c, sb_ht, sb_qt, sb_n90, sb_scl)
                b2mm.append(m0)
                ctx_mms(c - 1, pexp, att_ps)
                pexp = ex
                if c == 8:
                    w0 = head_dmas(0, gate=b2mm[0])
                    _prod0["r"] = produce(0, w0)
                    _prod0["w"] = w0
            ctx_mms(15, pexp, att_ps)

            # stage the non-critical prologue DMAs behind early branch-2 compute
            for dma, gate in [
                (ht_dmas[1], b2mm[0]), (ht_dmas[2], b2mm[4]), (ht_dmas[3], b2mm[8]),
                (an_dmas[1], b2mm[2]), (an_dmas[2], b2mm[6]), (an_dmas[3], b2mm[10]),
                (dma_w1, b2mm[12]), (dma_w2, b2mm[12]),
            ]:
                add_dep_helper(dma.ins, gate.ins)

            for qb2 in range(4):
                rcol = colsp.tile([P, 1], F32, tag="cols", name=f"arc{qb2}")
                nc.vector.reciprocal(rcol, att_ps[qb2][:, S : S + 1])
                nc.vector.tensor_scalar_mul(
                    sb_attn[:, qb2, :], att_ps[qb2][:, 0:S], rcol
                )

            def head_normalize(h, ctx_ps):
                # normalize by the softmax denominators (ones-column); emitting
                # this before produce(h+1) releases the psB banks ASAP
                sb_ctx = ctxp.tile([P, 4, S], F32, tag="ctx", name=f"ctxs{h}")
                for qb2 in range(4):
                    rcol = colsp.tile([P, 1], F32, tag="cols", name=f"crc{h}_{qb2}")
                    nc.vector.reciprocal(rcol, ctx_ps[qb2][:, S : S + 1])
                    nc.vector.tensor_scalar_mul(
                        sb_ctx[:, qb2, :], ctx_ps[qb2][:, 0:S], rcol
                    )
                return sb_ctx

            def head_tail(h, w, sb_ctx, sb_wct):
                # bvo partial: bias contribution wo_h @ bv_h (N=2, zero-padded)
                bps = psA.tile([P, NQ], F32, tag="work", name=f"bvps{h}")
                for ms in range(2):
                    for fc in range(2):
                        nc.tensor.matmul(
                            bps[:, ms * 2 : ms * 2 + 2],
                            w["o"][:, fc, ms * P : (ms + 1) * P],
                            sb_bvz[:, h * 2 + fc, :],
                            start=(fc == 0), stop=(fc == 1),
                        )
                for ms in range(2):
                    nc.vector.tensor_add(
                        sb_boeff[:, ms : ms + 1], sb_boeff[:, ms : ms + 1],
                        bps[:, ms * 2 : ms * 2 + 1],
                    )
                sb_ctxt = ctxp.tile([P, 2, NQ], MMDT, tag="ctxt", name=f"ctxt{h}")
                for m in range(2):
                    for qb2 in range(4):
                        pst = psA.tile([P, NQ], F32, tag="work", name=f"tp{h}_{m}_{qb2}")
                        nc.tensor.transpose(
                            pst[:, 0:P], sb_ctx[:, qb2, m * P : (m + 1) * P], ident
                        )
                        nc.scalar.copy(
                            sb_ctxt[:, m, qb2 * P : (qb2 + 1) * P], pst[:, 0:P]
                        )
                # A_mT partial for this head, accumulated into SBUF
                for ms in range(2):
                    ps = psA.tile([P, NQ], F32, tag="work", name=f"amp{h}_{ms}")
                    for sic in range(2):
                        nc.tensor.matmul(
                            ps,
                            sb_wct[:, sic, ms * P : (ms + 1) * P],
                            sb_ctxt[:, sic, :],
                            start=(sic == 0), stop=(sic == 1),
                        )
                    nc.vector.tensor_add(sb_amt[:, ms, :], sb_amt[:, ms, :], ps)

            sb_qwt, sb_wct = _prod0["r"]
            w = _prod0["w"]
            for h in range(NH):
                wn = head_dmas(h + 1, gate=None) if h + 1 < NH else None
                ctx_ps = [psB.tile([P, S + 2], F32, tag="acc", name=f"ctxps{h}_{i}") for i in range(4)]
                pexp, _ = sc_exp(f"h{h}", 0, sb_ht, sb_qwt, 0.0, 1.0)
                for c in range(1, 16):
                    ex, _ = sc_exp(f"h{h}", c, sb_ht, sb_qwt, 0.0, 1.0)
                    ctx_mms(c - 1, pexp, ctx_ps)
                    pexp = ex
                ctx_mms(15, pexp, ctx_ps)
                this_w, this_wct = w, sb_wct
                sb_ctx = head_normalize(h, ctx_ps)
                if h + 1 < NH:
                    sb_qwt, sb_wct = produce(h + 1, wn)
                    w = wn
                head_tail(h, this_w, sb_ctx, this_wct)

            # ============ A_m + attn_out, LayerNorm, FFN, LayerNorm ============
            for ms in range(2):
                nc.vector.tensor_scalar_add(
                    sb_amt[:, ms, :], sb_amt[:, ms, :], sb_boeff[:, ms : ms + 1]
                )

            sb_sum = ctxp.tile([P, 4, S], F32, tag="ctx")

            def layernorm_tile(y, x, tag):
                # y = (x - mean)/sqrt(var + eps) * g + b   for one [P, S] tile
                st = colsp.tile([P, 6], F32, tag="bn6", name=f"st_{tag}")
                nc.vector.bn_stats(st, x)
                mv = colsp.tile([P, 2], F32, tag="bn2", name=f"mv_{tag}")
                nc.vector.bn_aggr(mv, st)
                sq = colsp.tile([P, 1], F32, tag="cols", name=f"sq_{tag}")
                nc.scalar.activation(sq, mv[:, 1:2], Sqrt, bias=sb_eps, scale=1.0)
                rst = colsp.tile([P, 1], F32, tag="cols", name=f"rs_{tag}")
                nc.vector.reciprocal(rst, sq)
                nc.vector.tensor_scalar(y, x, mv[:, 0:1], rst, SUB, MUL)
                nc.vector.tensor_mul(y, y, sb_g)
                nc.vector.tensor_add(y, y, sb_b)

            sb_ad = ctxp.tile([P, 4, S], F32, tag="ad")
            sb_adt = ctxp.tile([P, 2, NQ], MMDT, tag="ctxt")
            sb_ff2t = ctxp.tile([P, 2, NQ], F32, tag="f2t")
            sb_y = ctxp.tile([P, 4, S], F32, tag="ctx", name="sb_y")
            sb_o = ctxp.tile([P, 4, S], F32, tag="ad", name="sb_o")
            out_r = d_out.rearrange("(qb p) s -> p qb s", p=P)

            for qb2 in range(4):
                q_sl = slice(qb2 * P, (qb2 + 1) * P)
                # merge A_m^T (transposed back) with attn_out
                for ms in range(2):
                    pst = psA.tile([P, NQ], F32, tag="work", name=f"tam{ms}_{qb2}")
                    nc.tensor.transpose(
                        pst[:, 0:P], sb_amt[:, ms, q_sl], ident
                    )
                    nc.vector.tensor_add(
                        sb_sum[:, qb2, ms * P : (ms + 1) * P],
                        pst[:, 0:P],
                        sb_attn[:, qb2, ms * P : (ms + 1) * P],
                    )
                layernorm_tile(sb_ad[:, qb2, :], sb_sum[:, qb2, :], f"a{qb2}")
                for ms in range(2):
                    pst = psA.tile([P, NQ], F32, tag="work", name=f"tad{ms}_{qb2}")
                    nc.tensor.transpose(
                        pst[:, 0:P], sb_ad[:, qb2, ms * P : (ms + 1) * P], ident
                    )
                    nc.scalar.copy(sb_adt[:, ms, q_sl], pst[:, 0:P])
                for hb in range(4):
                    ps = psB.tile([P, S + 2], F32, tag="acc", name=f"f1ps{hb}_{qb2}")
                    for ei in range(2):
                        nc.tensor.matmul(
                            ps[:, 0:P],
                            sb_w1t[:, ei, hb * P : (hb + 1) * P],
                            sb_adt[:, ei, q_sl],
                            start=(ei == 0), stop=(ei == 1),
                        )
                    nc.scalar.activation(
                        sb_ff1t[:, hb, q_sl], ps[:, 0:P], Relu,
                        bias=sb_b1c[:, hb : hb + 1], scale=1.0,
                    )
                for ms in range(2):
                    ps = psB.tile([P, S + 2], F32, tag="acc", name=f"f2ps{ms}_{qb2}")
                    for hc in range(4):
                        nc.tensor.matmul(
                            ps[:, 0:P],
                            sb_w2t[:, hc, ms * P : (ms + 1) * P],
                            sb_ff1t[:, hc, q_sl],
                            start=(hc == 0), stop=(hc == 3),
                        )
                    nc.scalar.activation(
                        sb_ff2t[:, ms, q_sl], ps[:, 0:P], Iden,
                        bias=sb_b2c[:, ms : ms + 1], scale=1.0,
                    )
                for ms in range(2):
                    pst = psA.tile([P, NQ], F32, tag="work", name=f"tf{ms}_{qb2}")
                    nc.tensor.transpose(
                        pst[:, 0:P], sb_ff2t[:, ms, q_sl], ident
                    )
                    nc.vector.tensor_add(
                        sb_y[:, qb2, ms * P : (ms + 1) * P],
                        pst[:, 0:P],
                        sb_ad[:, qb2, ms * P : (ms + 1) * P],
                    )
                layernorm_tile(sb_o[:, qb2, :], sb_y[:, qb2, :], f"o{qb2}")
                nc.sync.dma_start(out_r[:, qb2, :], sb_o[:, qb2, :])

    nc.compile()
    return nc


def make_in_maps(inputs):
    """Host-side sharding: slicing/transpose/reshape/constant-padding only."""
    f = lambda a: np.ascontiguousarray(np.asarray(a, dtype=np.float32))
    Q, H, A = f(inputs["Q"]), f(inputs["H"]), f(inputs["A"])
    wq, wk, wv, wo = f(inputs["wq"]), f(inputs["wk"]), f(inputs["wv"]), f(inputs["wo"])
    w1, w2 = f(inputs["w1"]), f(inputs["w2"])
    bq, bv, bo = f(inputs["bq"]), f(inputs["bv"]), f(inputs["bo"])
    b1, b2 = f(inputs["b1"]), f(inputs["b2"])
    ln_g, ln_b = f(inputs["ln_g"]), f(inputs["ln_b"])
    scale = np.full((P, 1), np.float32(np.asarray(inputs["attn_scale"])), np.float32)

    bvz = np.zeros((P, 16, 2), np.float32)
    bvz[:, :, 0] = bv.reshape(16, P).T

    shared = {
        "wqt": f(wq.T), "wkn": wk, "wvn": wv, "wot": f(wo.T),
        "w1t": f(w1.T), "w2t": f(w2.T),
        "bqc": f(bq.reshape(16, P).T), "bvz": bvz,
        "boc": f(bo.reshape(2, P).T),
        "b1c": f(b1.reshape(4, P).T), "b2c": f(b2.reshape(2, P).T),
        "gr": f(ln_g.reshape(1, S)), "br": f(ln_b.reshape(1, S)),
        "scl": scale,
    }
    in_maps = []
    for core in range(NCORES):
        b, qb = core // 4, core % 4
        m = dict(shared)
        m["qt"] = f(Q[b, qb * NQ : (qb + 1) * NQ, :].T)
        m["ht"] = f(H[b].T)
        pad = np.zeros((SK, 2), np.float32)
        pad[:, 0] = 1.0
        m["anat"] = f(np.concatenate([A[b], pad], axis=1))
        in_maps.append(m)
    return in_maps


def _install_ntff_hook_shim():
    """Provide antenv.axon_hooks (absent in this image) so trace=True works."""
    import sys as _sys
    import types as _types

    if "antenv.axon_hooks" in _sys.modules:
        return True
    try:
        from trn_agent_boot.trn_boot import _ntff_profile_via_ctypes

        hook = _ntff_profile_via_ctypes("/opt/axon/libaxon_pjrt.so")
        if hook is None:
            return False
        mod = _types.ModuleType("antenv.axon_hooks")
        mod._hook = hook
        mod.get_axon_ntff_profile_hook = lambda: mod._hook
        mod.set_axon_ntff_profile_hook = lambda h: setattr(mod, "_hook", h)
        _sys.modules["antenv.axon_hooks"] = mod
        import antenv

        antenv.axon_hooks = mod
        return True
    except Exception:
        return False


def kernel(**inputs) -> np.ndarray:
    global LAST_RESULT
    nc = build_nc()
    in_maps = make_in_maps(inputs)
    trace = os.environ.get("BASS_PROFILE", "0") == "1"
    if trace:
        trace = _install_ntff_hook_shim()
    res = run_bass_kernel_spmd(nc, in_maps, core_ids=list(range(NCORES)), trace=trace)
    LAST_RESULT = res
    out = np.empty((B, SQ, S), dtype=np.float32)
    for core in range(NCORES):
        b, qb = core // 4, core % 4
        out[b, qb * NQ : (qb + 1) * NQ, :] = res.results[core]["out"]
    return out


if __name__ == "__main__":
    nc = build_nc()
    print("build ok")


# revision 19
# speedup vs baseline: 1.0753x; 1.0753x over previous
"""Trainium2 Bass kernel for DeductionNetworkSingleLayer.

Sharding: data-parallel over (batch, query-block). 8 cores; core c handles
batch b = c // 4, query rows [qb*512, (qb+1)*512) with qb = c % 4.
Each core computes the full network for its 512 query rows; no collectives.

Algebraic restructuring (all exact reassociations):
  - scoresT_h = (H wk_h^T q_h^T)^T is computed as H @ (wk_h^T q_h^T), so the
    per-head K projection over the full 2048-key sequence collapses into a
    256x512 "qw" matrix. bk drops out entirely (softmax shift-invariance
    over keys; only the query-side bias bq affects the distribution).
  - ctx_h = probs_h @ (A wv_h^T + bv) is computed as (probs_h @ [A|1|0]),
    with wv and wo merged into one per-head matrix wcomb_h = wo_h @ wv_h
    (built on-chip once per head), and the bv term reduced to the constant
    bias wo @ bv + bo applied to the accumulated A_m. The ones column of the
    augmented A yields the softmax denominator from the same matmul; the
    zero column pads N to an even count (fp32r requirement).

The head loop is software-pipelined: head h+1's q/qw/wcomb production is
emitted between head h's key-block loop and its tail, so the PE never waits
on the production's eviction chains. Prologue DMAs are staged with explicit
dependency edges so the first-needed chunks get full HBM bandwidth.

Host-side prep is pure layout marshalling (slicing / transposes / reshapes /
constant padding, no arithmetic).
"""

import os
import sys

import numpy as np

for _p in ("/opt/trn_rl_repo", os.path.expanduser("~/.axon_site/_ro/trn_rl_repo")):
    if _p not in sys.path and os.path.isdir(_p):
        sys.path.insert(0, _p)

import concourse.bass as bass
import concourse.mybir as mybir
import concourse.tile as tile
from concourse import bacc
from concourse.bass_utils import run_bass_kernel_spmd
from concourse.masks import make_identity
from concourse.tile import add_dep_helper

P = 128
B, SQ, SK = 2, 2048, 2048
E = 256          # embed dim == per-head key dim
S = 256          # src dim == per-head value dim
NH = 8
HID = 2 * S      # 512
NQ = 512         # query rows per core
NCORES = 8
EXP2_SHIFT = -90.0  # constant softmax shift for the raw-QK branch
F32 = mybir.dt.float32

LAST_RESULT = None


def _bcast_row(row_ap, parts=P):
    """AP that broadcasts a [1, N] DRAM row across `parts` partitions."""
    return bass.AP(
        tensor=row_ap.tensor,
        offset=row_ap.offset,
        ap=[[0, parts]] + list(row_ap.ap)[1:],
    )


def build_nc(mm_dtype_name: str | None = None):
    """Build the Bass program (same SPMD program for all 8 cores)."""
    MMDT = getattr(mybir.dt, mm_dtype_name or os.environ.get("BASS_MM_DTYPE", "float32r"))

    nc = bacc.Bacc("TRN2", target_bir_lowering=False, debug=False)

    di = lambda name, shape, dt=F32: nc.dram_tensor(name, shape, dt, kind="ExternalInput").ap()
    d_qt = di("qt", [E, NQ], MMDT)        # Q-shard transposed
    d_ht = di("ht", [E, SK], MMDT)        # H[b] transposed
    d_anat = di("anat", [SK, S + 2], MMDT)  # A[b] | ones | zeros
    d_wqt = di("wqt", [E, NH * E], MMDT)  # wq.T
    d_wkn = di("wkn", [NH * E, E], MMDT)  # wk (natural)
    d_wvn = di("wvn", [NH * S, S], MMDT)  # wv (natural)
    d_wot = di("wot", [NH * S, S], MMDT)  # wo.T
    d_w1t = di("w1t", [S, HID], MMDT)
    d_w2t = di("w2t", [HID, S], MMDT)
    d_bqc = di("bqc", [P, 16])            # bq as [128,16] column chunks
    d_bvz = di("bvz", [P, 16, 2], MMDT)   # bv col chunks | zeros (even-N rhs)
    d_boc = di("boc", [P, 2])
    d_b1c = di("b1c", [P, 4])
    d_b2c = di("b2c", [P, 2])
    d_gr = di("gr", [1, S])               # ln_g row
    d_br = di("br", [1, S])               # ln_b row
    d_scl = di("scl", [P, 1])             # attn_scale broadcast column
    d_out = nc.dram_tensor("out", [NQ, S], F32, kind="ExternalOutput").ap()

    with tile.TileContext(nc) as tc:
        from contextlib import ExitStack

        with ExitStack() as ctx:
            singles = ctx.enter_context(tc.tile_pool(name="singles", bufs=1))
            wts = ctx.enter_context(tc.tile_pool(name="wts", bufs=2))
            qthp = ctx.enter_context(tc.tile_pool(name="qthp", bufs=2))
            expp = ctx.enter_context(tc.tile_pool(name="expp", bufs=4))
            ctxp = ctx.enter_context(tc.tile_pool(name="ctxp", bufs=2))
            colsp = ctx.enter_context(tc.tile_pool(name="colsp", bufs=8))
            psA = ctx.enter_context(tc.tile_pool(name="psA", bufs=4, space="PSUM"))
            psB = ctx.enter_context(tc.tile_pool(name="psB", bufs=4, space="PSUM"))

            # -------- prologue loads; critical chunks first, rest dep-gated ----
            sb_qt = singles.tile([P, 2, NQ], MMDT, tag="qt")
            qt_r = d_qt.rearrange("(e p) n -> p e n", p=P)
            sb_ht = singles.tile([P, 2, SK], MMDT, tag="ht")
            ht_r = d_ht.rearrange("(e p) n -> p e n", p=P)
            # first-needed pieces get dedicated (small) transfers
            nc.sync.dma_start(sb_qt[:, 0:1, :], qt_r[:, 0:1, :])
            nc.sync.dma_start(
                sb_ht[:, 0:1, 0:512], ht_r[:, 0:1, 0:512]
            )
            nc.sync.dma_start(sb_qt[:, 1:2, :], qt_r[:, 1:2, :])
            ht_dmas = [None]
            nc.sync.dma_start(
                sb_ht[:, 1:2, 0:512], ht_r[:, 1:2, 0:512]
            )
            for nb in range(1, 4):
                ht_dmas.append(nc.sync.dma_start(
                    sb_ht[:, :, nb * 512 : (nb + 1) * 512],
                    ht_r[:, :, nb * 512 : (nb + 1) * 512],
                ))
            sb_anat = singles.tile([P, 16, S + 2], MMDT, tag="anat")
            an_r = d_anat.rearrange("(c p) s -> p c s", p=P)
            an_dmas = []
            for nb in range(4):
                an_dmas.append(nc.sync.dma_start(
                    sb_anat[:, nb * 4 : (nb + 1) * 4, :],
                    an_r[:, nb * 4 : (nb + 1) * 4, :],
                ))
            sb_w1t = singles.tile([P, 2, HID], MMDT, tag="w1t")
            dma_w1 = nc.sync.dma_start(sb_w1t, d_w1t.rearrange("(e p) n -> p e n", p=P))
            sb_w2t = singles.tile([P, 4, S], MMDT, tag="w2t")
            dma_w2 = nc.sync.dma_start(sb_w2t, d_w2t.rearrange("(t p) s -> p t s", p=P))

            sb_bqc = singles.tile([P, 16], F32, tag="bqc")
            nc.sync.dma_start(sb_bqc, d_bqc)
            sb_bvz = singles.tile([P, 16, 2], MMDT, tag="bvz")
            nc.sync.dma_start(sb_bvz, d_bvz)
            sb_boc = singles.tile([P, 2], F32, tag="boc")
            nc.sync.dma_start(sb_boc, d_boc)
            sb_b1c = singles.tile([P, 4], F32, tag="b1c")
            nc.sync.dma_start(sb_b1c, d_b1c)
            sb_b2c = singles.tile([P, 2], F32, tag="b2c")
            nc.sync.dma_start(sb_b2c, d_b2c)
            sb_scl = singles.tile([P, 1], F32, tag="scl")
            nc.sync.dma_start(sb_scl, d_scl)
            sb_g = singles.tile([P, S], F32, tag="gbc")
            nc.gpsimd.dma_start(sb_g, _bcast_row(d_gr[0:1, :]))
            sb_b = singles.tile([P, S], F32, tag="bbc")
            nc.gpsimd.dma_start(sb_b, _bcast_row(d_br[0:1, :]))

            ident = singles.tile([P, P], F32, tag="ident")
            make_identity(nc, ident)
            sb_n90 = singles.tile([P, 1], F32, tag="n90")
            nc.gpsimd.memset(sb_n90, EXP2_SHIFT)
            sb_eps = singles.tile([P, 1], F32, tag="eps")
            nc.gpsimd.memset(sb_eps, 1e-5)

            # bq pre-scaled by 1/16 (q-projection eviction computes (x+bq)/16)
            sb_bq16 = singles.tile([P, 16], F32, tag="bq16")
            nc.vector.tensor_scalar_mul(sb_bq16, sb_bqc, 1.0 / 16.0)
            sb_attn = singles.tile([P, 4, S], F32, tag="attn")
            sb_amt = singles.tile([P, 2, NQ], F32, tag="amt")
            nc.gpsimd.memset(sb_amt, 0.0)
            sb_ff1t = singles.tile([P, 4, NQ], MMDT, tag="ff1t")
            sb_boeff = singles.tile([P, 2], F32, tag="boeff")
            nc.vector.tensor_copy(sb_boeff, sb_boc)

            Exp = mybir.ActivationFunctionType.Exp
            Iden = mybir.ActivationFunctionType.Identity
            Relu = mybir.ActivationFunctionType.Relu
            Sqrt = mybir.ActivationFunctionType.Sqrt
            Square = mybir.ActivationFunctionType.Square
            SUB = mybir.AluOpType.subtract
            MUL = mybir.AluOpType.mult
            ADD = mybir.AluOpType.add

            wot_r = d_wot.rearrange("(t p) s -> p t s", p=P)
            wqt_r = d_wqt.rearrange("(e p) n -> p e n", p=P)
            wkn_r = d_wkn.rearrange("(t p) e -> p t e", p=P)
            wvn_r = d_wvn.rearrange("(t p) s -> p t s", p=P)

            def sc_exp(tag, c, lhs_tile, rhs_tile, bias, scale):
                """scoresT block c + exp eviction (two halves for latency)."""
                ps = psA.tile([P, NQ], F32, tag="work", name=f"scps_{tag}_{c}")
                mm0 = nc.tensor.matmul(
                    ps, lhs_tile[:, 0, c * P : (c + 1) * P], rhs_tile[:, 0, :],
                    start=True, stop=False,
                )
                nc.tensor.matmul(
                    ps, lhs_tile[:, 1, c * P : (c + 1) * P], rhs_tile[:, 1, :],
                    start=False, stop=True,
                )
                ex = expp.tile([P, NQ], MMDT, tag="exp", name=f"exp_{tag}_{c}")
                nc.scalar.activation(ex[:, 0:256], ps[:, 0:256], Exp, bias=bias, scale=scale)
                nc.scalar.activation(ex[:, 256:512], ps[:, 256:512], Exp, bias=bias, scale=scale)
                return ex, mm0

            def ctx_mms(c, ex, acc):
                for qb2 in range(4):
                    nc.tensor.matmul(
                        acc[qb2],
                        ex[:, qb2 * P : (qb2 + 1) * P],
                        sb_anat[:, c, :],
                        start=(c == 0),
                        stop=(c == 15),
                    )

            # ============ Branch 1: 8-head attention (software-pipelined) ========
            def head_dmas(h, gate=None):
                w = {}
                w["q"] = wts.tile([P, 2, E], MMDT, tag="wq", name=f"wqh{h}")
                d1 = nc.sync.dma_start(w["q"], wqt_r[:, :, h * E : (h + 1) * E])
                w["k"] = wts.tile([P, 2, E], MMDT, tag="wk", name=f"wkh{h}")
                d2 = nc.sync.dma_start(w["k"], wkn_r[:, h * 2 : h * 2 + 2, :])
                w["v"] = wts.tile([P, 2, S], MMDT, tag="wv", name=f"wvh{h}")
                d3 = nc.sync.dma_start(w["v"], wvn_r[:, h * 2 : h * 2 + 2, :])
                w["o"] = wts.tile([P, 2, S], MMDT, tag="wo", name=f"woh{h}")
                d4 = nc.sync.dma_start(w["o"], wot_r[:, h * 2 : h * 2 + 2, :])
                if gate is not None:
                    for d in (d1, d2, d3, d4):
                        add_dep_helper(d.ins, gate.ins)
                return w

            def produce(h, w):
                """qth, wct, qwt for head h (wct between the two dependent steps)."""
                sb_qth = qthp.tile([P, 2, NQ], MMDT, tag="qth", name=f"qth{h}")
                qps = []
                for eo in range(2):
                    ps = psA.tile([P, NQ], F32, tag="work", name=f"qps{h}_{eo}")
                    for ei in range(2):
                        nc.tensor.matmul(
                            ps,
                            w["q"][:, ei, eo * P : (eo + 1) * P],
                            sb_qt[:, ei, :],
                            start=(ei == 0), stop=(ei == 1),
                        )
                    qps.append(ps)
                # wcombT_h = wv_h^T @ wo_h^T (independent; fills the evict gap)
                sb_wct = ctxp.tile([P, 2, S], MMDT, tag="wct", name=f"wct{h}")
                for sb2 in range(2):
                    ps = psA.tile([P, NQ], F32, tag="work", name=f"wcps{h}_{sb2}")
                    for fc in range(2):
                        nc.tensor.matmul(
                            ps[:, 0:S],
                            w["v"][:, fc, sb2 * P : (sb2 + 1) * P],
                            w["o"][:, fc, :],
                            start=(fc == 0), stop=(fc == 1),
                        )
                    nc.scalar.copy(sb_wct[:, sb2, :], ps[:, 0:S])
                for eo in range(2):
                    nc.vector.tensor_scalar(
                        sb_qth[:, eo, :], qps[eo], 1.0 / 16.0,
                        sb_bq16[:, h * 2 + eo : h * 2 + eo + 1], MUL, ADD,
                    )
                sb_qwt = qthp.tile([P, 2, NQ], MMDT, tag="qwt", name=f"qwt{h}")
                for eo in range(2):
                    ps = psA.tile([P, NQ], F32, tag="work", name=f"qwps{h}_{eo}")
                    for fc in range(2):
                        nc.tensor.matmul(
                            ps,
                            w["k"][:, fc, eo * P : (eo + 1) * P],
                            sb_qth[:, fc, :],
                            start=(fc == 0), stop=(fc == 1),
                        )
                    nc.vector.tensor_copy(sb_qwt[:, eo, :], ps)
                return sb_qwt, sb_wct

            # ============ Branch 2: attn_out = softmax(Q H^T * scale) @ A ========
            att_ps = [psB.tile([P, S + 2], F32, tag="acc", name=f"attps{i}") for i in range(4)]
            b2mm = []
            _prod0 = {}
            pexp, m0 = sc_exp("b2", 0, sb_ht, sb_qt, sb_n90, sb_scl)
            b2mm.append(m0)
            for c in range(1, 16):
                ex, m0 = sc_exp("b2", c, sb_ht, sb_qt, sb_n90, sb_scl)
                b2mm.append(m0)
                ctx_mms(c - 1, pexp, att_ps)
                pexp = ex
                if c == 8:
                    w0 = head_dmas(0, gate=b2mm[0])
                    _prod0["r"] = produce(0, w0)
                    _prod0["w"] = w0
            ctx_mms(15, pexp, att_ps)

            # stage the non-critical prologue DMAs behind early branch-2 compute
            for dma, gate in [
                (ht_dmas[1], b2mm[0]), (ht_dmas[2], b2mm[4]), (ht_dmas[3], b2mm[8]),
                (an_dmas[1], b2mm[2]), (an_dmas[2], b2mm[6]), (an_dmas[3], b2mm[10]),
                (dma_w1, b2mm[12]), (dma_w2, b2mm[12]),
            ]:
                add_dep_helper(dma.ins, gate.ins)

            for qb2 in range(4):
                rcol = colsp.tile([P, 1], F32, tag="cols", name=f"arc{qb2}")
                nc.vector.reciprocal(rcol, att_ps[qb2][:, S : S + 1])
                nc.vector.tensor_scalar_mul(
                    sb_attn[:, qb2, :], att_ps[qb2][:, 0:S], rcol
                )

            def head_normalize(h, ctx_ps):
                # normalize by the softmax denominators (ones-column); emitting
                # this before produce(h+1) releases the psB banks ASAP
                sb_ctx = ctxp.tile([P, 4, S], F32, tag="ctx", name=f"ctxs{h}")
                for qb2 in range(4):
                    rcol = colsp.tile([P, 1], F32, tag="cols", name=f"crc{h}_{qb2}")
                    nc.vector.reciprocal(rcol, ctx_ps[qb2][:, S : S + 1])
                    nc.vector.tensor_scalar_mul(
                        sb_ctx[:, qb2, :], ctx_ps[qb2][:, 0:S], rcol
                    )
                return sb_ctx

            def head_tail(h, w, sb_ctx, sb_wct):
                # bvo partial: bias contribution wo_h @ bv_h (N=2, zero-padded)
                bps = psA.tile([P, NQ], F32, tag="work", name=f"bvps{h}")
                for ms in range(2):
                    for fc in range(2):
                        nc.tensor.matmul(
                            bps[:, ms * 2 : ms * 2 + 2],
                            w["o"][:, fc, ms * P : (ms + 1) * P],
                            sb_bvz[:, h * 2 + fc, :],
                            start=(fc == 0), stop=(fc == 1),
                        )
                for ms in range(2):
                    nc.vector.tensor_add(
                        sb_boeff[:, ms : ms + 1], sb_boeff[:, ms : ms + 1],
                        bps[:, ms * 2 : ms * 2 + 1],
                    )
                sb_ctxt = ctxp.tile([P, 2, NQ], MMDT, tag="ctxt", name=f"ctxt{h}")
                for m in range(2):
                    for qb2 in range(4):
                        pst = psA.tile([P, NQ], F32, tag="work", name=f"tp{h}_{m}_{qb2}")
                        nc.tensor.transpose(
                            pst[:, 0:P], sb_ctx[:, qb2, m * P : (m + 1) * P], ident
                        )
                        nc.scalar.copy(
                            sb_ctxt[:, m, qb2 * P : (qb2 + 1) * P], pst[:, 0:P]
                        )
                # A_mT partial for this head, accumulated into SBUF
                for ms in range(2):
                    ps = psA.tile([P, NQ], F32, tag="work", name=f"amp{h}_{ms}")
                    for sic in range(2):
                        nc.tensor.matmul(
                            ps,
                            sb_wct[:, sic, ms * P : (ms + 1) * P],
                            sb_ctxt[:, sic, :],
                            start=(sic == 0), stop=(sic == 1),
                        )
                    nc.vector.tensor_add(sb_amt[:, ms, :], sb_amt[:, ms, :], ps)

            sb_qwt, sb_wct = _prod0["r"]
            w = _prod0["w"]
            for h in range(NH):
                wn = head_dmas(h + 1, gate=None) if h + 1 < NH else None
                ctx_ps = [psB.tile([P, S + 2], F32, tag="acc", name=f"ctxps{h}_{i}") for i in range(4)]
                pexp, _ = sc_exp(f"h{h}", 0, sb_ht, sb_qwt, 0.0, 1.0)
                for c in range(1, 16):
                    ex, _ = sc_exp(f"h{h}", c, sb_ht, sb_qwt, 0.0, 1.0)
                    ctx_mms(c - 1, pexp, ctx_ps)
                    pexp = ex
                ctx_mms(15, pexp, ctx_ps)
                this_w, this_wct = w, sb_wct
                sb_ctx = head_normalize(h, ctx_ps)
                if h + 1 < NH:
                    sb_qwt, sb_wct = produce(h + 1, wn)
                    w = wn
                head_tail(h, this_w, sb_ctx, this_wct)

            # ============ A_m + attn_out, LayerNorm, FFN, LayerNorm ============
            for ms in range(2):
                nc.vector.tensor_scalar_add(
                    sb_amt[:, ms, :], sb_amt[:, ms, :], sb_boeff[:, ms : ms + 1]
                )

            sb_sum = ctxp.tile([P, 4, S], F32, tag="ctx")

            def layernorm_tile(y, x, tag):
                # y = (x - mean)/sqrt(var + eps) * g + b   for one [P, S] tile
                st = colsp.tile([P, 6], F32, tag="bn6", name=f"st_{tag}")
                nc.vector.bn_stats(st, x)
                mv = colsp.tile([P, 2], F32, tag="bn2", name=f"mv_{tag}")
                nc.vector.bn_aggr(mv, st)
                sq = colsp.tile([P, 1], F32, tag="cols", name=f"sq_{tag}")
                nc.scalar.activation(sq, mv[:, 1:2], Sqrt, bias=sb_eps, scale=1.0)
                rst = colsp.tile([P, 1], F32, tag="cols", name=f"rs_{tag}")
                nc.vector.reciprocal(rst, sq)
                nc.vector.tensor_scalar(y, x, mv[:, 0:1], rst, SUB, MUL)
                nc.vector.tensor_mul(y, y, sb_g)
                nc.vector.tensor_add(y, y, sb_b)

            sb_ad = ctxp.tile([P, 4, S], F32, tag="ad")
            for ms in range(2):
                for qb2 in range(4):
                    pst = psA.tile([P, NQ], F32, tag="work", name=f"tam{ms}_{qb2}")
                    nc.tensor.transpose(
                        pst[:, 0:P], sb_amt[:, ms, qb2 * P : (qb2 + 1) * P], ident
                    )
                    nc.vector.tensor_add(
                        sb_sum[:, qb2, ms * P : (ms + 1) * P],
                        pst[:, 0:P],
                        sb_attn[:, qb2, ms * P : (ms + 1) * P],
                    )
            for qb2 in range(4):
                layernorm_tile(sb_ad[:, qb2, :], sb_sum[:, qb2, :], f"a{qb2}")

            sb_adt = ctxp.tile([P, 2, NQ], MMDT, tag="ctxt")
            for ms in range(2):
                for qb2 in range(4):
                    pst = psA.tile([P, NQ], F32, tag="work", name=f"tad{ms}_{qb2}")
                    nc.tensor.transpose(
                        pst[:, 0:P], sb_ad[:, qb2, ms * P : (ms + 1) * P], ident
                    )
                    nc.scalar.copy(sb_adt[:, ms, qb2 * P : (qb2 + 1) * P], pst[:, 0:P])

            for hb in range(4):
                ps = psB.tile([P, NQ], F32, tag="acc", name=f"f1ps{hb}")
                for ei in range(2):
                    nc.tensor.matmul(
                        ps,
                        sb_w1t[:, ei, hb * P : (hb + 1) * P],
                        sb_adt[:, ei, :],
                        start=(ei == 0), stop=(ei == 1),
                    )
                nc.scalar.activation(
                    sb_ff1t[:, hb, :], ps, Relu, bias=sb_b1c[:, hb : hb + 1], scale=1.0
                )

            sb_ff2t = ctxp.tile([P, 2, NQ], F32, tag="ctxt")
            for ms in range(2):
                ps = psB.tile([P, NQ], F32, tag="acc", name=f"f2ps{ms}")
                for hc in range(4):
                    nc.tensor.matmul(
                        ps,
                        sb_w2t[:, hc, ms * P : (ms + 1) * P],
                        sb_ff1t[:, hc, :],
                        start=(hc == 0), stop=(hc == 3),
                    )
                nc.scalar.activation(
                    sb_ff2t[:, ms, :], ps, Iden, bias=sb_b2c[:, ms : ms + 1], scale=1.0
                )

            sb_y = ctxp.tile([P, 4, S], F32, tag="ctx", name="sb_y")
            sb_o = ctxp.tile([P, 4, S], F32, tag="ad", name="sb_o")
            out_r = d_out.rearrange("(qb p) s -> p qb s", p=P)
            for ms in range(2):
                for qb2 in range(4):
                    pst = psA.tile([P, NQ], F32, tag="work", name=f"tf{ms}_{qb2}")
                    nc.tensor.transpose(
                        pst[:, 0:P], sb_ff2t[:, ms, qb2 * P : (qb2 + 1) * P], ident
                    )
                    nc.vector.tensor_add(
                        sb_y[:, qb2, ms * P : (ms + 1) * P],
                        pst[:, 0:P],
                        sb_ad[:, qb2, ms * P : (ms + 1) * P],
                    )
            for qb2 in range(4):
                layernorm_tile(sb_o[:, qb2, :], sb_y[:, qb2, :], f"o{qb2}")
                nc.sync.dma_start(out_r[:, qb2, :], sb_o[:, qb2, :])

    nc.compile()
    return nc


def make_in_maps(inputs):
    """Host-side sharding: slicing/transpose/reshape/constant-padding only."""
    f = lambda a: np.ascontiguousarray(np.asarray(a, dtype=np.float32))
    Q, H, A = f(inputs["Q"]), f(inputs["H"]), f(inputs["A"])
    wq, wk, wv, wo = f(inputs["wq"]), f(inputs["wk"]), f(inputs["wv"]), f(inputs["wo"])
    w1, w2 = f(inputs["w1"]), f(inputs["w2"])
    bq, bv, bo = f(inputs["bq"]), f(inputs["bv"]), f(inputs["bo"])
    b1, b2 = f(inputs["b1"]), f(inputs["b2"])
    ln_g, ln_b = f(inputs["ln_g"]), f(inputs["ln_b"])
    scale = np.full((P, 1), np.float32(np.asarray(inputs["attn_scale"])), np.float32)

    bvz = np.zeros((P, 16, 2), np.float32)
    bvz[:, :, 0] = bv.reshape(16, P).T

    shared = {
        "wqt": f(wq.T), "wkn": wk, "wvn": wv, "wot": f(wo.T),
        "w1t": f(w1.T), "w2t": f(w2.T),
        "bqc": f(bq.reshape(16, P).T), "bvz": bvz,
        "boc": f(bo.reshape(2, P).T),
        "b1c": f(b1.reshape(4, P).T), "b2c": f(b2.reshape(2, P).T),
        "gr": f(ln_g.reshape(1, S)), "br": f(ln_b.reshape(1, S)),
        "scl": scale,
    }
    in_maps = []
    for core in range(NCORES):
        b, qb = core // 4, core % 4
        m = dict(shared)
        m["qt"] = f(Q[b, qb * NQ : (qb + 1) * NQ, :].T)
        m["ht"] = f(H[b].T)
        pad = np.zeros((SK, 2), np.float32)
        pad[:, 0] = 1.0
        m["anat"] = f(np.concatenate([A[b], pad], axis=1))
        in_maps.append(m)
    return in_maps


def _install_ntff_hook_shim():
    """Provide antenv.axon_hooks (absent in this image) so trace=True works."""
    import sys as _sys
    import types as _types

    if "antenv.axon_hooks" in _sys.modules:
        return True
    try:
        from trn_agent_boot.trn_boot import _ntff_profile_via_ctypes

        hook = _ntff_profile_via_ctypes("/opt/axon/libaxon_pjrt.so")
        if hook is None:
            return False
        mod = _types.ModuleType("antenv.axon_hooks")
        mod._hook = hook
        mod.get_axon_ntff_profile_hook = lambda: mod._hook
        mod.set_axon_ntff_profile_hook = lambda h: setattr(mod, "_hook", h)
        _sys.modules["antenv.axon_hooks"] = mod
        import antenv

        antenv.axon_hooks = mod
        return True
    except Exception:
        return False


def kernel(**inputs) -> np.ndarray:
    global LAST_RESULT
    nc = build_nc()
    in_maps = make_in_maps(inputs)
    trace = os.environ.get("BASS_PROFILE", "0") == "1"
    if trace:
        trace = _install_ntff_hook_shim()
    res = run_bass_kernel_spmd(nc, in_maps, core_ids=list(range(NCORES)), trace=trace)
    LAST_RESULT = res
    out = np.empty((B, SQ, S), dtype=np.float32)
    for core in range(NCORES):
        b, qb = core // 4, core % 4
        out[b, qb * NQ : (qb + 1) * NQ, :] = res.results[core]["out"]
    return out


if __name__ == "__main__":
    nc = build_nc()
    print("build ok")
